# revision 57
# baseline (speedup 1.0000x reference)
"""NetVLAD (vq_codebook) Trainium2 Bass kernel, 8-way spatially sharded.

Math (validated in numpy + CoreSim to rel ~2e-3 vs reference, gate 2e-2):
  xn = x / ||x||_C per location (host); logits = conv_w @ xn; soft = softmax_K
  fold(unfold(soft) * top2keep) == soft * cnt, cnt = 3x3 box-sum of the
  per-cluster top-2 indicator. vlad = sa2 @ xn.T - rowsum(sa2) * centroids,
  then intra + global L2 norm (host).

vs the fp32 baseline (174us -> ~63us HW):
- fp8e4m3 x in both layouts + conv weights + w2 (mask host-rescaled into
  fp8 range; the global scale cancels in the final L2 norms); bf16
  elsewhere; fp32 PSUM accumulation.
- col-tiled K=64 matmuls: two concurrent 64-col groups in the 128x128 PE
  array (logits c-tile pairs, VLAD l-tile pairs; host sums the halves).
- logits transposed+halves-summed by a regular matmul against a stacked
  identity; exp batched 8 tiles per PSUM bank.
- top-2 via a tournament (hi/lo chains) in DVE 2x mode, two L stripes so
  stripe A's keep-transposes overlap stripe B's chain.
- 3x3 box-sum on a 128-partition packed buffer (two L halves stacked in
  the partition dim via paired keep-transposes), separable h/v passes,
  shifted kb copy for 4B alignment.
- sustained PE warm-up bursts (HAM K=8/8) over the DMA-bound start and
  the box phase; packed small inputs; stream-first DMA order.

Sharding: H=192 rows split 8 ways (24 rows/core + 1 halo row each side).
[128, C+1] partial VLAD sums summed on host across col groups and cores.
"""
import os
import sys

sys.path.insert(0, "/opt/trn_rl_repo")
os.environ.setdefault("MYCRO_LOCAL_CACHE", "1")

import numpy as np

C, H, W, K = 512, 192, 192, 64
M = 8                      # cores
RPC = H // M               # 24 rows per core
Ls = (RPC + 2) * W         # 4992 slab locations (incl. 1 halo row each side)
NT = Ls // 128             # 39 l-tiles
CT = C // 128              # 4 c-tiles
G1 = 257                   # kb guard (odd -> v-pass offsets even for DVE 2x)
PKW = 22 * 128             # 2816 packed box-sum columns (22 tiles)
KBW = G1 + PKW + G1        # 3330 keep-buffer width
POF = 17                   # B row-group holds tiles 17..38 (l offset 2176)
NPAIR = 22                 # keep-T pair transposes (t, t+17), t=0..21
XW = 8                     # xnt DMA batching (tiles per DMA wave)

# cnt-T schedule: pairs (j, j+17) j=3..19 (A rows->tile j, B rows->tile j+17),
# then singles: tiles 0..2 from A rows, tiles 37..38 from B rows.
CNT_PAIRS = list(range(3, 20))
CNT_SINGLE_A = [0, 1, 2]
CNT_SINGLE_B = [37, 38]
# VLAD slot order = w2 availability order (earliest cnt2-column need
# first, so the tail starts right after box chunk 0); host permutes xnT
# rows to match.
WORK_ITEMS = ([("s", t) for t in CNT_SINGLE_A]
              + [("p", j) for j in CNT_PAIRS]
              + [("s", t) for t in CNT_SINGLE_B])
SLOT_TILES = []
for _k, _v in WORK_ITEMS:
    SLOT_TILES += [_v, _v + POF] if _k == "p" else [_v]   # 39 slots

TRACE = False              # set by test.py for profiling runs
_CACHE = {}


def _build_nc():
    import concourse.bass as bass
    import concourse.tile as tile
    from concourse import mybir

    f32 = mybir.dt.float32
    bf16 = mybir.dt.bfloat16
    AF = mybir.ActivationFunctionType
    OP = mybir.AluOpType
    AX = mybir.AxisListType

    fp8 = mybir.dt.float8e4
    nc = bass.Bass()
    xnb = nc.dram_tensor("xnb", [C, Ls], fp8, kind="ExternalInput")
    xnt = nc.dram_tensor("xnt", [Ls, C], bf16, kind="ExternalInput")
    # small inputs packed into two tensors (fewer serialized DMA triggers):
    # smallb = identb(128) | mstack(64) | msk-as-bf16(NT) cols, bf16
    # small8 = cwt(4*64) | ones8(8) cols, fp8
    smallb = nc.dram_tensor("smallb", [128, 128 + K + NT], bf16,
                            kind="ExternalInput")
    small8 = nc.dram_tensor("small8", [128, CT * K + 8], fp8,
                            kind="ExternalInput")
    y = nc.dram_tensor("y", [128, C + 1], f32, kind="ExternalOutput")

    with tile.TileContext(nc) as tc:
        with tc.tile_pool(name="big", bufs=1) as big:
            # persistent SBUF tensors
            xnb_sb = big.tile([128, CT * Ls], fp8, tag="xnb")
            xnt_sb = big.tile([128, NT * C], bf16, tag="xnt")
            logklb = big.tile([128, Ls], bf16, tag="logklb")
            expb = big.tile([128, NT * K], bf16, tag="expb")
            tmpb = big.tile([128, NT * K], bf16, tag="tmpb")
            keep2 = big.tile([128, NPAIR * 128], bf16, tag="keep2")
            kb = big.tile([128, KBW], bf16, tag="kb")
            kb2 = big.tile([128, KBW], bf16, tag="kb2")
            h3s = big.tile([128, KBW], bf16, tag="h3s")
            cnt2 = big.tile([128, PKW], bf16, tag="cnt2")
            cnt_lk = big.tile([128, NPAIR * 128 + 5 * K], bf16, tag="cntlk")
            w2 = big.tile([128, NT * K], bf16, tag="w2")
            trs = big.tile([128, NT * K], bf16, tag="trs")
            trs2 = big.tile([128, NT * 32], bf16, tag="trs2")
            smallb_sb = big.tile([128, 128 + K + NT], bf16, tag="smallb")
            small8_sb = big.tile([128, CT * K + 8], fp8, tag="small8")
            id_sb = smallb_sb[:, 0:128]
            mst_sb = smallb_sb[:, 128:128 + K]
            msk_sb = smallb_sb[:, 128 + K:128 + K + NT]
            cwt_sb = small8_sb[:, 0:CT * K]
            ones8_sb = small8_sb[:, CT * K:CT * K + 8]
            warm = big.tile([128, 512], bf16, tag="warm")
            sume = big.tile([128, NT], f32, tag="sume")
            m1b = big.tile([128, NT], f32, tag="m1b")
            m2b = big.tile([128, NT], f32, tag="m2b")
            isum = big.tile([128, NT], f32, tag="isum")
            scc = big.tile([128, NT], f32, tag="scc")
            vl_sb = big.tile([128, C + 1], f32, tag="vl")
            scr = big.tile([128, 4], f32, tag="scr")

            # zero the keep-buffer guards (both row groups); build the PE
            # warm-up operand without any DMA dependency
            nc.vector.memset(kb[:, 0:G1], 0.0)
            nc.vector.memset(kb[:, G1 + PKW:KBW], 0.0)
            nc.vector.memset(warm[:], 1.0)

            # DMA order: first xnb chunk first (it gates the logits), then
            # the two packed small inputs, then the remaining chunks
            xc3 = xnb[:].rearrange("(ct p) l -> p ct l", p=128)
            xs3 = xnb_sb[:].rearrange("p (ct l) -> p ct l", l=Ls)
            DCH = 6
            csz = Ls // DCH              # 832 cols per chunk, all c-tiles
            nc.sync.dma_start(xs3[:, :, 0:csz], xc3[:, :, 0:csz])
            nc.sync.dma_start(smallb_sb[:], smallb[:])
            nc.sync.dma_start(small8_sb[:], small8[:])
            for j in range(1, DCH):
                nc.sync.dma_start(
                    xs3[:, :, j * csz:(j + 1) * csz],
                    xc3[:, :, j * csz:(j + 1) * csz],
                )
            # touch ops absorbing the packed-small-DMA completions so
            # downstream compute carries at most one sync wait each
            nc.scalar.copy(scr[:, 0:1], msk_sb[:, 0:1])
            nc.vector.tensor_copy(scr[:, 1:2], msk_sb[:, 1:2])

            with tc.tile_pool(name="pp", bufs=1, space="PSUM") as pp:
                pv0 = pp.tile([128, C], f32, tag="pv0", bufs=1)
                pv1 = pp.tile([128, 8], f32, tag="pv1", bufs=1)
                # sustained warm-up burst (~4us of PE activity, no DMA deps):
                # trips the HAM to K=8/8 during the DMA-bound startup so the
                # logits matmuls run at 2.4 GHz from the first block
                for i in range(10):
                    dummy = pp.tile([128, 512], f32, tag="pm", bufs=2)
                    nc.tensor.matmul(dummy[0:64, :], lhsT=warm[:, 0:64],
                                     rhs=warm[:], start=True, stop=True)
                # absorb the packed small-input DMA sems on the PE stream
                dummy = pp.tile([128, 512], f32, tag="pm", bufs=2)
                nc.tensor.matmul(dummy[0:64, 0:64], lhsT=cwt_sb[:, 0:64],
                                 rhs=cwt_sb[:, 0:64], start=True, stop=True)
                nc.tensor.matmul(dummy[0:64, 0:64], lhsT=id_sb[:, 0:64],
                                 rhs=mst_sb[:, 0:64], start=True, stop=True)

                # ---- phase 1: logits [K, L] col-tiled (ct0/ct2 -> psum rows
                # 0:64, ct1/ct3 -> rows 64:128), streamed over xnb chunks
                nblk = (Ls + 511) // 512
                touched = set()
                for b in range(nblk):
                    w = min(512, Ls - b * 512)
                    for j in range((b * 512) // csz,
                                   (b * 512 + w - 1) // csz + 1):
                        if j not in touched:
                            touched.add(j)
                            dj = pp.tile([128, 512], f32, tag="pm", bufs=2)
                            nc.tensor.matmul(
                                dj[0:64, 0:64],
                                lhsT=xnb_sb[:, j * csz:j * csz + 64],
                                rhs=xnb_sb[:, j * csz:j * csz + 64],
                                start=True, stop=True)
                    plog = pp.tile([128, 512], f32, tag="pm", bufs=2)
                    for ct in range(CT):
                        nc.tensor.matmul(
                            plog[(ct % 2) * 64:(ct % 2) * 64 + 64, 0:w],
                            lhsT=cwt_sb[:, ct * K:(ct + 1) * K],
                            rhs=xnb_sb[:, ct * Ls + b * 512:
                                       ct * Ls + b * 512 + w],
                            start=(ct < 2),
                            stop=(ct >= 2),
                            tile_position=(0, (ct % 2) * 64),
                            skip_group_check=True,
                        )
                    nc.scalar.copy(logklb[:, b * 512:b * 512 + w],
                                   plog[:, 0:w])

                # ---- phase 2: transpose logits tiles with Mstack (sums the
                # two col-group partials) then exp; 8 tiles batched per PSUM
                # bank so one activation covers 8 tiles
                t = 0
                while t < NT:
                    n = min(8, NT - t)
                    pt = pp.tile([128, 8 * K], f32, tag="pt", bufs=4)
                    for i in range(n):
                        # regular matmul: out = logklb_tile.T @ Mstack sums
                        # the two col-group partials while transposing
                        nc.tensor.matmul(
                            pt[:, i * K:(i + 1) * K],
                            lhsT=logklb[:, (t + i) * 128:(t + i + 1) * 128],
                            rhs=mst_sb[:], start=True, stop=True,
                            skip_group_check=True)
                    nc.scalar.activation(
                        expb[:, t * K:(t + n) * K], pt[:, 0:n * K], AF.Exp)
                    t += n

                # ---- phase 3: batched top-2 + softmax denominators
                e3 = expb[:].rearrange("p (t k) -> p t k", k=K)
                m2bc = m2b[:][:, :, None].broadcast_to([128, NT, K])
                k4 = keep2[:].rearrange("p (t k) -> p t k", k=128)
                hiv = tmpb[:].rearrange("p (t k) -> p t k", k=K)
                lov = trs[:].rearrange("p (t k) -> p t k", k=K)
                tt2 = trs2[:].rearrange("p (t k) -> p t k", k=32)

                def ham_keep(src):
                    # small matmul reading `src`: keeps the PE HAM warm during
                    # the DVE-heavy phases and absorbs that engine's sem on
                    # the PE stream (later PE waits become prunable)
                    n = min(src.shape[-1], 64)
                    dh = pp.tile([128, 512], f32, tag="pm", bufs=2)
                    nc.tensor.matmul(dh[0:n, 0:n], lhsT=src[:, 0:n],
                                     rhs=src[:, 0:n], start=True, stop=True)

                def top2_stripe(ts, te):
                    # tournament second-max: hi = block max, lo = block
                    # second-max; merging two blocks:
                    #   hi' = max(hi_l, hi_r)
                    #   lo' = max(min(hi_l, hi_r), max(lo_l, lo_r))
                    # all stages run in DVE 2x mode (bf16, aligned halves)
                    nc.vector.tensor_tensor(
                        hiv[:, ts:te, 0:32], e3[:, ts:te, 0:32],
                        e3[:, ts:te, 32:64], op=OP.max)
                    nc.vector.tensor_tensor(
                        lov[:, ts:te, 0:32], e3[:, ts:te, 0:32],
                        e3[:, ts:te, 32:64], op=OP.min)
                    off, w = 0, 32
                    while w > 1:
                        nw = w // 2
                        noff = off + w
                        hl = hiv[:, ts:te, off:off + nw]
                        hr = hiv[:, ts:te, off + nw:off + w]
                        ll = lov[:, ts:te, off:off + nw]
                        lr = lov[:, ts:te, off + nw:off + w]
                        t1 = tt2[:, ts:te, 0:nw]
                        t2 = tt2[:, ts:te, nw:2 * nw]
                        nc.vector.tensor_tensor(t1, hl, hr, op=OP.min)
                        nc.vector.tensor_tensor(t2, ll, lr, op=OP.max)
                        if nw == 1:
                            nc.vector.tensor_tensor(
                                m2b[:, ts:te][:, :, None], t1, t2, op=OP.max)
                        else:
                            nc.vector.tensor_tensor(
                                lov[:, ts:te, noff:noff + nw], t1, t2,
                                op=OP.max)
                            nc.vector.tensor_tensor(
                                hiv[:, ts:te, noff:noff + nw], hl, hr,
                                op=OP.max)
                        off, w = noff, nw

                # two L stripes so stripe A's keep-T/kb work overlaps
                # stripe B's top-2 chain (stripe A = tiles 0..21, B = 22..38)
                for (ts, te) in ((0, NPAIR), (NPAIR, NT)):
                    top2_stripe(ts, te)
                    ham_keep(tmpb[:, 0:32])
                    if ts == 0:
                        # keep pair-slot cols 0:64 = tiles 0..21 (stripe A)
                        # and cols 64:128 of slots 0..4 = tiles 17..21
                        nc.vector.tensor_tensor(
                            k4[:, :, 0:K], e3[:, 0:NPAIR], m2bc[:, 0:NPAIR],
                            op=OP.is_ge)
                        nc.vector.tensor_tensor(
                            k4[:, 0:5, K:128], e3[:, POF:POF + 5],
                            m2bc[:, POF:POF + 5], op=OP.is_ge)
                    else:
                        # cols 64:128 of slots 5..21 = tiles 22..38 (B)
                        nc.vector.tensor_tensor(
                            k4[:, 5:NPAIR, K:128], e3[:, POF + 5:NT],
                            m2bc[:, POF + 5:NT], op=OP.is_ge)

                # ---- phase 4: keep-T pair transposes into the packed
                # guarded buffer: kb rows 0:64 = keep[K, l] for l tiles 0..21,
                # rows 64:128 = tiles 17..38, same columns
                for t in range(NPAIR):
                    pk = pp.tile([128, 128], bf16, tag="pt", bufs=4)
                    nc.tensor.transpose(
                        pk[:], keep2[:, t * 128:(t + 1) * 128], id_sb[:])
                    nc.scalar.copy(kb[:, G1 + t * 128:G1 + (t + 1) * 128],
                                   pk[:])
                    if t % 6 == 5:   # keep the PE HAM warm through this phase
                        dh = pp.tile([128, 512], f32, tag="pm", bufs=2)
                        nc.tensor.matmul(
                            dh[0:64, 0:64], lhsT=warm[:, 0:64],
                            rhs=warm[:, 0:64], start=True, stop=True)

                # softmax denominators off the critical chain (run on DVE
                # while the PE does keep-T transposes); halving add-tree in
                # 2x mode beats the all-1x tensor_reduce
                nc.vector.tensor_add(
                    hiv[:, :, 0:32], e3[:, :, 0:32], e3[:, :, 32:64])
                off, w = 0, 32
                while w > 1:
                    nw = w // 2
                    noff = off + w
                    dst = (sume[:][:, :, None] if nw == 1
                           else hiv[:, :, noff:noff + nw])
                    nc.vector.tensor_add(
                        dst, hiv[:, :, off:off + nw],
                        hiv[:, :, off + nw:off + w])
                    off, w = noff, nw
                nc.vector.reciprocal(isum[:], sume[:])
                nc.vector.tensor_mul(scc[:], msk_sb[:], isum[:])

                # ---- phase 5: separable 3x3 box-sum on the packed
                # buffer, in two column chunks so chunk 0 starts while late
                # keep-T copies are still landing. kb2 = kb shifted by one
                # (4x-mode copy) so all h-pass taps are 4B-aligned (DVE 2x).
                # h3s[j] = kb[j] + kb[j+1] + kb[j+2]  (h[j+1], shifted)
                # cnt[c] = h[G1+c-192] + h[G1+c] + h[G1+c+192], h = h3s[j-1]
                HSPLIT = 1856
                VSPLIT = 1408
                for ci in range(2):
                    h0, h1 = (0, HSPLIT) if ci == 0 else (HSPLIT, KBW - 2)
                    c0, c1 = (0, VSPLIT) if ci == 0 else (VSPLIT, PKW)
                    nc.vector.tensor_copy(kb2[:, h0:h1], kb[:, h0 + 1:h1 + 1])
                    nc.vector.tensor_add(
                        h3s[:, h0:h1], kb[:, h0:h1], kb[:, h0 + 2:h1 + 2])
                    nc.vector.tensor_add(
                        h3s[:, h0:h1], h3s[:, h0:h1], kb2[:, h0:h1])
                    nc.vector.tensor_add(
                        cnt2[:, c0:c1], h3s[:, G1 - 193 + c0:G1 - 193 + c1],
                        h3s[:, G1 + 191 + c0:G1 + 191 + c1])
                    nc.vector.tensor_add(
                        cnt2[:, c0:c1], cnt2[:, c0:c1],
                        h3s[:, G1 - 1 + c0:G1 - 1 + c1])
                    if ci == 0:
                        # sustained re-warm burst riding the rest of the box
                        # phase so the VLAD tail starts at K=8/8
                        dh = pp.tile([128, 512], f32, tag="pm", bufs=2)
                        nc.tensor.matmul(
                            dh[0:64, 0:64], lhsT=h3s[:, 0:64],
                            rhs=h3s[:, 0:64], start=True, stop=True)
                        for _ in range(9):
                            dh = pp.tile([128, 512], f32, tag="pm", bufs=2)
                            nc.tensor.matmul(
                                dh[0:64, :], lhsT=warm[:, 0:64], rhs=warm[:],
                                start=True, stop=True)
                    else:
                        ham_keep(cnt2[:, c0:c0 + 64])

                # ---- phase 6: cnt-T back to [l, K], fuse w2 = cnt*scc*exp,
                # and immediately accumulate VLAD for each finished tile.
                # xnT arrives permuted in VLAD slot order (5 waves).
                x3 = xnt[:].rearrange("(a p) c -> p a c", p=128)
                xt3 = xnt_sb[:].rearrange("p (a c) -> p a c", c=C)
                nwav = (NT + XW - 1) // XW
                for wv in range(nwav):
                    n = min(XW, NT - wv * XW)
                    nc.sync.dma_start(
                        xt3[:, wv * XW:wv * XW + n, :],
                        x3[:, wv * XW:wv * XW + n, :],
                    )

                slot = 0
                started = [False, False]   # col group A (tiles<20), B

                def vlad_mm(tl, last):
                    nonlocal slot
                    grp = 0 if tl < 20 else 1
                    rows = slice(grp * 64, grp * 64 + 64)
                    lt = w2[:, tl * K:(tl + 1) * K]
                    if slot % XW == 0:     # absorb this wave's DMA wait
                        dw = pp.tile([128, 512], f32, tag="pm", bufs=2)
                        nc.tensor.matmul(
                            dw[0:64, 0:64],
                            lhsT=xnt_sb[:, slot * C:slot * C + 64],
                            rhs=xnt_sb[:, slot * C:slot * C + 64],
                            start=True, stop=True)
                    nc.tensor.matmul(
                        pv0[rows, :], lhsT=lt,
                        rhs=xnt_sb[:, slot * C:(slot + 1) * C],
                        start=not started[grp], stop=last,
                        tile_position=(0, grp * 64),
                        skip_group_check=True)
                    nc.tensor.matmul(
                        pv1[rows, 0:1], lhsT=lt, rhs=warm[:, 0:1],
                        start=not started[grp], stop=last,
                        tile_position=(0, grp * 64),
                        skip_group_check=True)
                    started[grp] = True
                    slot += 1

                def w2_fuse(tl, src, eng=None):
                    # alternate DVE / gpsimd so neither engine gates the tail
                    (eng or nc.vector).scalar_tensor_tensor(
                        w2[:, tl * K:(tl + 1) * K], src,
                        scc[:, tl:tl + 1], expb[:, tl * K:(tl + 1) * K],
                        op0=OP.mult, op1=OP.mult)

                # work items ordered by cnt2-column need; transposes are
                # emitted 3 ahead of their consumers (pt bufs=4) so the PE
                # never stalls on the act/DVE pipeline behind it
                cl_off = [0]
                for kind, v in WORK_ITEMS:
                    cl_off.append(cl_off[-1] + (128 if kind == "p" else K))

                def emit_T(i):
                    kind, v = WORK_ITEMS[i]
                    pc = pp.tile([128, 128], bf16, tag="pt", bufs=4)
                    if kind == "p":
                        nc.tensor.transpose(
                            pc[:], cnt2[:, v * 128:(v + 1) * 128], id_sb[:])
                    elif v < 20:
                        nc.tensor.transpose(
                            pc[:, 0:K], cnt2[0:64, v * 128:(v + 1) * 128],
                            id_sb[0:64, 0:K])
                    else:
                        nc.tensor.transpose(
                            pc[:, 0:K],
                            cnt2[64:128,
                                 (v - POF) * 128:(v - POF + 1) * 128],
                            id_sb[64:128, 64:64 + K])
                    return pc

                def consume(i, pc):
                    kind, v = WORK_ITEMS[i]
                    if kind == "p":
                        cl = cnt_lk[:, cl_off[i]:cl_off[i] + 128]
                        nc.scalar.copy(cl, pc[:])
                        tA, tB = v, v + POF
                        w2_fuse(tA, cl[:, 0:K])
                        w2_fuse(tB, cl[:, K:128])
                        vlad_mm(tA, last=(tA == 19))
                        vlad_mm(tB, last=False)
                    else:
                        cl = cnt_lk[:, cl_off[i]:cl_off[i] + K]
                        nc.scalar.copy(cl, pc[:, 0:K])
                        w2_fuse(v, cl)
                        vlad_mm(v, last=(v == 38))

                NW = len(WORK_ITEMS)
                pend = []
                for i in range(min(3, NW)):
                    pend.append(emit_T(i))
                for i in range(NW):
                    consume(i, pend[i])
                    if i + 3 < NW:
                        pend.append(emit_T(i + 3))

                # ---- phase 7: write this core's [128, C+1] partial sums;
                # host sums col groups + cores, applies centroid subtraction
                # and the two L2 normalizations
                nc.scalar.copy(vl_sb[:, 0:C], pv0[:])
                nc.scalar.copy(vl_sb[:, C:C + 1], pv1[:, 0:1])
                nc.sync.dma_start(y[:], vl_sb[:])
    _prune_waits(nc)
    return nc


def _prune_waits(nc):
    """Drop semaphore waits that are transitively implied by another wait on
    the same instruction (walrus codegen allows one hw wait per compute
    instruction; extra waits cost separate EVENT_SEMAPHORE instructions)."""
    insts = [ins for bb in nc.main_func.blocks for ins in bb.instructions]
    proc_events = {}
    waits_of = {}
    pending = {}    # engine -> waits of non-ticking instrs (e.g. Ldweights),
    #                 folded into the next ticking instr on that engine so the
    #                 transitive closure can see them (engines run in-order)
    for ins in insts:
        si = getattr(ins, "sync_info", None)
        if si is None:
            continue
        eng = getattr(ins, "engine", None)
        ow = [(w.ant_name, w.wait_value) for w in (si.on_wait or [])]
        carried = pending.get(eng, [])
        all_waits = carried + ow
        ticked = False
        for u in (si.on_update or []):
            if getattr(u, "update_mode", None) not in ("sem-inc", "sem-add-imm"):
                continue
            ticked = True
            lst = proc_events.setdefault(u.ant_name, [])
            prev = lst[-1][0] if lst else 0
            lst.append((prev + (u.update_value or 1), ins))
        waits_of[id(ins)] = all_waits if ticked else ow
        pending[eng] = [] if ticked else all_waits

    import bisect

    def prefix_index(sem, v):
        lst = proc_events.get(sem)
        if not lst:
            return None
        ticks = [t for t, _ in lst]
        i = bisect.bisect_left(ticks, v)
        return i if i < len(lst) else None

    memo = {}

    def holds(sem, v, depth=0):
        if depth > 6:
            return {}
        i = prefix_index(sem, v)
        if i is None:
            return {}
        key = (sem, i)
        if key in memo:
            return memo[key]
        memo[key] = {}
        out = {}
        inorder = not sem.startswith("Pool")
        rng = range(i + 1) if inorder else (i,)
        for j in rng:
            _, ins = proc_events[sem][j]
            for (s2, v2) in waits_of.get(id(ins), []):
                if out.get(s2, 0) < v2:
                    out[s2] = v2
                sub = holds(s2, v2, depth + 1)
                for s3, v3 in sub.items():
                    if out.get(s3, 0) < v3:
                        out[s3] = v3
        memo[key] = out
        return out

    own_tick = {}
    for sem, lst in proc_events.items():
        for tick, ins in lst:
            own_tick[(id(ins), sem)] = tick

    pruned = 0
    for ins in insts:
        si = getattr(ins, "sync_info", None)
        if si is None or not si.on_wait or len(si.on_wait) < 2:
            continue
        ow = list(si.on_wait)
        kept = list(ow)
        tn = type(ins).__name__
        is_dma = "DMA" in tn or "Drain" in tn
        for w in ow:
            if len(kept) == 1:
                break
            # same-queue FIFO rule, DMA instructions only: waiting on earlier
            # completions of the queue this DMA executes on is vacuous
            # (per-queue serial execution). Compute engines keep such waits:
            # the race detector requires them when APs overlap.
            if is_dma:
                mine = own_tick.get((id(ins), w.ant_name))
                if mine is not None and w.wait_value <= mine - 1:
                    kept.remove(w)
                    pruned += 1
                    continue
            others = [o for o in kept if o is not w]
            for o in others:
                h = holds(o.ant_name, o.wait_value)
                if h.get(w.ant_name, 0) >= w.wait_value:
                    kept.remove(w)
                    pruned += 1
                    break
        si.on_wait = kept
    return pruned


def _host_prep(x, conv_w, centroids):
    from concourse import mybir
    bf16np = mybir.dt.np(mybir.dt.bfloat16)
    fp8np = mybir.dt.np(mybir.dt.float8e4)

    x = np.ascontiguousarray(x, dtype=np.float32)
    norm = np.sqrt((x.astype(np.float64) ** 2).sum(0))
    xn = (x / np.maximum(norm, 1e-12)).astype(np.float32)    # [C,H,W]
    ii = np.arange(H, dtype=np.float64)
    mi = np.minimum(H - 1 - ii, ii)
    m = np.minimum(mi[:, None], mi[None, :])
    m4 = m ** 4
    # rescale so w2 = msk*soft*cnt fits fp8e4m3 range; the global scale
    # cancels in the intra-cluster L2 normalization on the host
    msk_full = (m4 / m4.max()).astype(np.float32)            # [H,W]

    xn_pad = np.zeros((C, H + 2, W), np.float32)
    xn_pad[:, 1:H + 1] = xn
    msk_pad = np.zeros((H + 2, W), np.float32)
    msk_pad[1:H + 1] = msk_full

    # packed small inputs
    cwtb = conv_w.T.astype(np.float32).reshape(CT, 128, K)
    cwtb = np.ascontiguousarray(cwtb.transpose(1, 0, 2)).reshape(128, CT * K)
    small8 = np.zeros((128, CT * K + 8), np.float32)
    small8[:, 0:CT * K] = cwtb
    small8[:, CT * K:] = 1.0
    small8 = small8.astype(fp8np)
    identb = np.eye(128, dtype=np.float32)
    mstack = np.concatenate([np.eye(K), np.eye(K)], 0).astype(np.float32)
    slot = np.array(SLOT_TILES)

    in_maps = []
    for core in range(M):
        r0 = core * RPC
        slab = np.ascontiguousarray(
            xn_pad[:, r0:r0 + RPC + 2, :].reshape(C, Ls))
        mskc = msk_pad[r0:r0 + RPC + 2].reshape(Ls).copy()
        mskc[0:W] = 0.0
        mskc[(RPC + 1) * W:] = 0.0                           # halo rows -> 0
        xnT = np.ascontiguousarray(slab.T).astype(bf16np)    # [Ls, C]
        # permute l-tiles into VLAD slot order
        xnT_perm = np.ascontiguousarray(
            xnT.reshape(NT, 128, C)[slot].reshape(Ls, C))
        smallb = np.zeros((128, 128 + K + NT), np.float32)
        smallb[:, 0:128] = identb
        smallb[:, 128:128 + K] = mstack
        smallb[:, 128 + K:] = mskc.reshape(NT, 128).T
        in_maps.append({
            "xnb": slab.astype(fp8np),
            "xnt": xnT_perm,
            "smallb": smallb.astype(bf16np),
            "small8": small8,
        })
    return in_maps


def _ensure_ntff_hook():
    """Install the axon NTFF profile hook if the image's antenv lacks it."""
    import types
    try:
        from antenv.axon_hooks import get_axon_ntff_profile_hook  # noqa: F401
        return
    except ImportError:
        pass
    if "/root/.axon_site" not in sys.path:
        sys.path.insert(0, "/root/.axon_site")
    from trn_agent_boot.trn_boot import _ntff_profile_via_ctypes
    hook = _ntff_profile_via_ctypes("/opt/axon/libaxon_pjrt.so")
    mod = types.ModuleType("antenv.axon_hooks")
    mod.get_axon_ntff_profile_hook = lambda: hook
    mod.set_axon_ntff_profile_hook = lambda h: None
    import antenv
    antenv.axon_hooks = mod
    sys.modules["antenv.axon_hooks"] = mod


def _install_neff_cache():
    """Cache compiled NEFFs across processes, keyed by BIR content hash."""
    import hashlib
    import shutil
    import concourse.bass2jax as b2j

    orig = b2j.compile_bir_kernel
    if getattr(orig, "_neff_cached", False):
        return

    def cached(bir_json, tmpdir, neff_name="file.neff"):
        h = hashlib.sha256(
            bir_json if isinstance(bir_json, bytes) else bir_json.encode()
        ).hexdigest()[:24]
        cdir = "/tmp/neff_cache"
        os.makedirs(cdir, exist_ok=True)
        cpath = os.path.join(cdir, h + ".neff")
        if os.path.exists(cpath):
            dst = os.path.join(tmpdir, neff_name)
            os.makedirs(tmpdir, exist_ok=True)
            shutil.copy(cpath, dst)
            return dst
        out = orig(bir_json, tmpdir, neff_name=neff_name)
        shutil.copy(out, cpath)
        return out

    cached._neff_cached = True
    b2j.compile_bir_kernel = cached


def kernel(x, conv_w, centroids):
    import concourse.bass_utils as bu
    from concourse.bass_utils import run_bass_kernel_spmd
    _install_neff_cache()
    if TRACE:
        _ensure_ntff_hook()
        bu.upload_artifacts = lambda tmpdir: "local://" + tmpdir

    if "nc" not in _CACHE:
        _CACHE["nc"] = _build_nc()
    nc = _CACHE["nc"]
    in_maps = _host_prep(np.asarray(x), np.asarray(conv_w), np.asarray(centroids))
    res = run_bass_kernel_spmd(nc, in_maps, list(range(M)), trace=TRACE)
    _CACHE["last"] = res
    red = np.zeros((128, C + 1), np.float64)
    for r in res.results:
        red += np.asarray(r["y"], dtype=np.float64)
    redk = red[0:64] + red[64:128]                           # [K, C+1]
    vlad = redk[:, :C] - redk[:, C:C + 1] * np.asarray(centroids, np.float64)
    vlad /= np.maximum(np.sqrt((vlad ** 2).sum(1))[:, None], 1e-12)
    v = vlad.reshape(1, K * C)
    v /= np.maximum(np.sqrt((v ** 2).sum()), 1e-12)
    return v.astype(np.float32)


# revision 59
# speedup vs baseline: 1.1019x; 1.1019x over previous
"""NetVLAD (vq_codebook) Trainium2 Bass kernel, 8-way spatially sharded.

Math (validated in numpy + CoreSim to rel ~2e-3 vs reference, gate 2e-2):
  xn = x / ||x||_C per location (host); logits = conv_w @ xn; soft = softmax_K
  fold(unfold(soft) * top2keep) == soft * cnt, cnt = 3x3 box-sum of the
  per-cluster top-2 indicator. vlad = sa2 @ xn.T - rowsum(sa2) * centroids,
  then intra + global L2 norm (host).

vs the fp32 baseline (174us -> ~63us HW):
- fp8e4m3 x in both layouts + conv weights + w2 (mask host-rescaled into
  fp8 range; the global scale cancels in the final L2 norms); bf16
  elsewhere; fp32 PSUM accumulation.
- col-tiled K=64 matmuls: two concurrent 64-col groups in the 128x128 PE
  array (logits c-tile pairs, VLAD l-tile pairs; host sums the halves).
- logits transposed+halves-summed by a regular matmul against a stacked
  identity; exp batched 8 tiles per PSUM bank.
- top-2 via a tournament (hi/lo chains) in DVE 2x mode, two L stripes so
  stripe A's keep-transposes overlap stripe B's chain.
- 3x3 box-sum on a 128-partition packed buffer (two L halves stacked in
  the partition dim via paired keep-transposes), separable h/v passes,
  shifted kb copy for 4B alignment.
- sustained PE warm-up bursts (HAM K=8/8) over the DMA-bound start and
  the box phase; packed small inputs; stream-first DMA order.

Sharding: H=192 rows split 8 ways (24 rows/core + 1 halo row each side).
[128, C+1] partial VLAD sums summed on host across col groups and cores.
"""
import os
import sys

sys.path.insert(0, "/opt/trn_rl_repo")
os.environ.setdefault("MYCRO_LOCAL_CACHE", "1")

import numpy as np

C, H, W, K = 512, 192, 192, 64
M = 8                      # cores
RPC = H // M               # 24 rows per core
Ls = (RPC + 2) * W         # 4992 slab locations (incl. 1 halo row each side)
NT = Ls // 128             # 39 l-tiles
CT = C // 128              # 4 c-tiles
G1 = 257                   # kb guard (odd -> v-pass offsets even for DVE 2x)
PKW = 22 * 128             # 2816 packed box-sum columns (22 tiles)
KBW = G1 + PKW + G1        # 3330 keep-buffer width
POF = 17                   # B row-group holds tiles 17..38 (l offset 2176)
NPAIR = 22                 # keep-T pair transposes (t, t+17), t=0..21
XW = 8                     # xnt DMA batching (tiles per DMA wave)

# cnt-T schedule: pairs (j, j+17) j=3..19 (A rows->tile j, B rows->tile j+17),
# then singles: tiles 0..2 from A rows, tiles 37..38 from B rows.
CNT_PAIRS = list(range(3, 20))
CNT_SINGLE_A = [0, 1, 2]
CNT_SINGLE_B = [37, 38]
# VLAD slot order = w2 availability order; host permutes xnT rows to match.
SLOT_TILES = []
for _j in CNT_PAIRS:
    SLOT_TILES += [_j, _j + POF]
SLOT_TILES += CNT_SINGLE_A + CNT_SINGLE_B        # 39 slots

TRACE = False              # set by test.py for profiling runs
_CACHE = {}


def _build_nc():
    import concourse.bass as bass
    import concourse.tile as tile
    from concourse import mybir

    f32 = mybir.dt.float32
    bf16 = mybir.dt.bfloat16
    AF = mybir.ActivationFunctionType
    OP = mybir.AluOpType
    AX = mybir.AxisListType

    fp8 = mybir.dt.float8e4
    nc = bass.Bass()
    xnb = nc.dram_tensor("xnb", [C, Ls], fp8, kind="ExternalInput")
    xnt = nc.dram_tensor("xnt", [Ls, C], bf16, kind="ExternalInput")
    # small inputs packed into two tensors (fewer serialized DMA triggers):
    # smallb = identb(128) | mstack(64) | msk-as-bf16(NT) cols, bf16
    # small8 = cwt(4*64) | ones8(8) cols, fp8
    smallb = nc.dram_tensor("smallb", [128, 128 + K + NT], bf16,
                            kind="ExternalInput")
    small8 = nc.dram_tensor("small8", [128, CT * K + 8], fp8,
                            kind="ExternalInput")
    y = nc.dram_tensor("y", [128, C + 1], f32, kind="ExternalOutput")

    with tile.TileContext(nc) as tc:
        with tc.tile_pool(name="big", bufs=1) as big:
            # persistent SBUF tensors
            xnb_sb = big.tile([128, CT * Ls], fp8, tag="xnb")
            xnt_sb = big.tile([128, NT * C], bf16, tag="xnt")
            logklb = big.tile([128, Ls], bf16, tag="logklb")
            expb = big.tile([128, NT * K], bf16, tag="expb")
            tmpb = big.tile([128, NT * K], bf16, tag="tmpb")
            keep2 = big.tile([128, NPAIR * 128], bf16, tag="keep2")
            kb = big.tile([128, KBW], bf16, tag="kb")
            kb2 = big.tile([128, KBW], bf16, tag="kb2")
            h3s = big.tile([128, KBW], bf16, tag="h3s")
            cnt2 = big.tile([128, PKW], bf16, tag="cnt2")
            cnt_lk = big.tile([128, NPAIR * 128 + 5 * K], bf16, tag="cntlk")
            w2 = big.tile([128, NT * K], bf16, tag="w2")
            trs = big.tile([128, NT * K], bf16, tag="trs")
            trs2 = big.tile([128, NT * 32], bf16, tag="trs2")
            smallb_sb = big.tile([128, 128 + K + NT], bf16, tag="smallb")
            small8_sb = big.tile([128, CT * K + 8], fp8, tag="small8")
            id_sb = smallb_sb[:, 0:128]
            mst_sb = smallb_sb[:, 128:128 + K]
            msk_sb = smallb_sb[:, 128 + K:128 + K + NT]
            cwt_sb = small8_sb[:, 0:CT * K]
            ones8_sb = small8_sb[:, CT * K:CT * K + 8]
            warm = big.tile([128, 512], bf16, tag="warm")
            sume = big.tile([128, NT], f32, tag="sume")
            m1b = big.tile([128, NT], f32, tag="m1b")
            m2b = big.tile([128, NT], f32, tag="m2b")
            isum = big.tile([128, NT], f32, tag="isum")
            scc = big.tile([128, NT], f32, tag="scc")
            vl_sb = big.tile([128, C + 1], f32, tag="vl")
            scr = big.tile([128, 4], f32, tag="scr")

            # zero the keep-buffer guards (both row groups); build the PE
            # warm-up operand without any DMA dependency
            nc.vector.memset(kb[:, 0:G1], 0.0)
            nc.vector.memset(kb[:, G1 + PKW:KBW], 0.0)
            nc.vector.memset(warm[:], 1.0)

            # DMA order: first xnb chunk first (it gates the logits), then
            # the two packed small inputs, then the remaining chunks
            xc3 = xnb[:].rearrange("(ct p) l -> p ct l", p=128)
            xs3 = xnb_sb[:].rearrange("p (ct l) -> p ct l", l=Ls)
            DCH = 4
            csz = Ls // DCH              # 1248 cols per chunk, all c-tiles
            nc.sync.dma_start(xs3[:, :, 0:csz], xc3[:, :, 0:csz])
            nc.sync.dma_start(smallb_sb[:], smallb[:])
            nc.sync.dma_start(small8_sb[:], small8[:])
            for j in range(1, DCH):
                nc.sync.dma_start(
                    xs3[:, :, j * csz:(j + 1) * csz],
                    xc3[:, :, j * csz:(j + 1) * csz],
                )
            # touch ops absorbing the packed-small-DMA completions so
            # downstream compute carries at most one sync wait each
            nc.scalar.copy(scr[:, 0:1], msk_sb[:, 0:1])
            nc.vector.tensor_copy(scr[:, 1:2], msk_sb[:, 1:2])

            with tc.tile_pool(name="pp", bufs=1, space="PSUM") as pp:
                pv0 = pp.tile([128, C], f32, tag="pv0", bufs=1)
                pv1 = pp.tile([128, 8], f32, tag="pv1", bufs=1)
                # sustained warm-up burst (~4us of PE activity, no DMA deps):
                # trips the HAM to K=8/8 during the DMA-bound startup so the
                # logits matmuls run at 2.4 GHz from the first block
                for i in range(8):
                    dummy = pp.tile([128, 512], f32, tag="pm", bufs=2)
                    nc.tensor.matmul(dummy[0:64, :], lhsT=warm[:, 0:64],
                                     rhs=warm[:], start=True, stop=True)
                # absorb the packed small-input DMA sems on the PE stream
                dummy = pp.tile([128, 512], f32, tag="pm", bufs=2)
                nc.tensor.matmul(dummy[0:64, 0:64], lhsT=cwt_sb[:, 0:64],
                                 rhs=cwt_sb[:, 0:64], start=True, stop=True)
                nc.tensor.matmul(dummy[0:64, 0:64], lhsT=id_sb[:, 0:64],
                                 rhs=mst_sb[:, 0:64], start=True, stop=True)

                # ---- phase 1: logits [K, L] col-tiled (ct0/ct2 -> psum rows
                # 0:64, ct1/ct3 -> rows 64:128), streamed over xnb chunks
                nblk = (Ls + 511) // 512
                touched = set()
                for b in range(nblk):
                    w = min(512, Ls - b * 512)
                    for j in range((b * 512) // csz,
                                   (b * 512 + w - 1) // csz + 1):
                        if j not in touched:
                            touched.add(j)
                            dj = pp.tile([128, 512], f32, tag="pm", bufs=2)
                            nc.tensor.matmul(
                                dj[0:64, 0:64],
                                lhsT=xnb_sb[:, j * csz:j * csz + 64],
                                rhs=xnb_sb[:, j * csz:j * csz + 64],
                                start=True, stop=True)
                    plog = pp.tile([128, 512], f32, tag="pm", bufs=2)
                    for ct in range(CT):
                        nc.tensor.matmul(
                            plog[(ct % 2) * 64:(ct % 2) * 64 + 64, 0:w],
                            lhsT=cwt_sb[:, ct * K:(ct + 1) * K],
                            rhs=xnb_sb[:, ct * Ls + b * 512:
                                       ct * Ls + b * 512 + w],
                            start=(ct < 2),
                            stop=(ct >= 2),
                            tile_position=(0, (ct % 2) * 64),
                            skip_group_check=True,
                        )
                    nc.scalar.copy(logklb[:, b * 512:b * 512 + w],
                                   plog[:, 0:w])

                # ---- phase 2: transpose logits tiles with Mstack (sums the
                # two col-group partials) then exp; 8 tiles batched per PSUM
                # bank so one activation covers 8 tiles
                t = 0
                while t < NT:
                    n = min(8, NT - t)
                    pt = pp.tile([128, 8 * K], f32, tag="pt", bufs=4)
                    for i in range(n):
                        # regular matmul: out = logklb_tile.T @ Mstack sums
                        # the two col-group partials while transposing
                        nc.tensor.matmul(
                            pt[:, i * K:(i + 1) * K],
                            lhsT=logklb[:, (t + i) * 128:(t + i + 1) * 128],
                            rhs=mst_sb[:], start=True, stop=True,
                            skip_group_check=True)
                    nc.scalar.activation(
                        expb[:, t * K:(t + n) * K], pt[:, 0:n * K], AF.Exp)
                    t += n

                # ---- phase 3: batched top-2 + softmax denominators
                e3 = expb[:].rearrange("p (t k) -> p t k", k=K)
                m2bc = m2b[:][:, :, None].broadcast_to([128, NT, K])
                k4 = keep2[:].rearrange("p (t k) -> p t k", k=128)
                hiv = tmpb[:].rearrange("p (t k) -> p t k", k=K)
                lov = trs[:].rearrange("p (t k) -> p t k", k=K)
                tt2 = trs2[:].rearrange("p (t k) -> p t k", k=32)

                def ham_keep(src):
                    # small matmul reading `src`: keeps the PE HAM warm during
                    # the DVE-heavy phases and absorbs that engine's sem on
                    # the PE stream (later PE waits become prunable)
                    n = min(src.shape[-1], 64)
                    dh = pp.tile([128, 512], f32, tag="pm", bufs=2)
                    nc.tensor.matmul(dh[0:n, 0:n], lhsT=src[:, 0:n],
                                     rhs=src[:, 0:n], start=True, stop=True)

                def top2_stripe(ts, te):
                    # tournament second-max: hi = block max, lo = block
                    # second-max; merging two blocks:
                    #   hi' = max(hi_l, hi_r)
                    #   lo' = max(min(hi_l, hi_r), max(lo_l, lo_r))
                    # all stages run in DVE 2x mode (bf16, aligned halves)
                    nc.vector.tensor_tensor(
                        hiv[:, ts:te, 0:32], e3[:, ts:te, 0:32],
                        e3[:, ts:te, 32:64], op=OP.max)
                    nc.vector.tensor_tensor(
                        lov[:, ts:te, 0:32], e3[:, ts:te, 0:32],
                        e3[:, ts:te, 32:64], op=OP.min)
                    off, w = 0, 32
                    while w > 1:
                        nw = w // 2
                        noff = off + w
                        hl = hiv[:, ts:te, off:off + nw]
                        hr = hiv[:, ts:te, off + nw:off + w]
                        ll = lov[:, ts:te, off:off + nw]
                        lr = lov[:, ts:te, off + nw:off + w]
                        t1 = tt2[:, ts:te, 0:nw]
                        t2 = tt2[:, ts:te, nw:2 * nw]
                        nc.vector.tensor_tensor(t1, hl, hr, op=OP.min)
                        nc.vector.tensor_tensor(t2, ll, lr, op=OP.max)
                        if nw == 1:
                            nc.vector.tensor_tensor(
                                m2b[:, ts:te][:, :, None], t1, t2, op=OP.max)
                        else:
                            nc.vector.tensor_tensor(
                                lov[:, ts:te, noff:noff + nw], t1, t2,
                                op=OP.max)
                            nc.vector.tensor_tensor(
                                hiv[:, ts:te, noff:noff + nw], hl, hr,
                                op=OP.max)
                        off, w = noff, nw

                # two L stripes so stripe A's keep-T/kb work overlaps
                # stripe B's top-2 chain (stripe A = tiles 0..21, B = 22..38)
                for (ts, te) in ((0, NPAIR), (NPAIR, NT)):
                    top2_stripe(ts, te)
                    ham_keep(tmpb[:, 0:32])
                    if ts == 0:
                        # keep pair-slot cols 0:64 = tiles 0..21 (stripe A)
                        # and cols 64:128 of slots 0..4 = tiles 17..21
                        nc.vector.tensor_tensor(
                            k4[:, :, 0:K], e3[:, 0:NPAIR], m2bc[:, 0:NPAIR],
                            op=OP.is_ge)
                        nc.vector.tensor_tensor(
                            k4[:, 0:5, K:128], e3[:, POF:POF + 5],
                            m2bc[:, POF:POF + 5], op=OP.is_ge)
                    else:
                        # cols 64:128 of slots 5..21 = tiles 22..38 (B)
                        nc.vector.tensor_tensor(
                            k4[:, 5:NPAIR, K:128], e3[:, POF + 5:NT],
                            m2bc[:, POF + 5:NT], op=OP.is_ge)

                # ---- phase 4: keep-T pair transposes into the packed
                # guarded buffer: kb rows 0:64 = keep[K, l] for l tiles 0..21,
                # rows 64:128 = tiles 17..38, same columns
                for t in range(NPAIR):
                    pk = pp.tile([128, 128], bf16, tag="pt", bufs=4)
                    nc.tensor.transpose(
                        pk[:], keep2[:, t * 128:(t + 1) * 128], id_sb[:])
                    nc.scalar.copy(kb[:, G1 + t * 128:G1 + (t + 1) * 128],
                                   pk[:])
                    if t % 6 == 5:   # keep the PE HAM warm through this phase
                        dh = pp.tile([128, 512], f32, tag="pm", bufs=2)
                        nc.tensor.matmul(
                            dh[0:64, 0:64], lhsT=warm[:, 0:64],
                            rhs=warm[:, 0:64], start=True, stop=True)

                # softmax denominators off the critical chain (run on DVE
                # while the PE does keep-T transposes); halving add-tree in
                # 2x mode beats the all-1x tensor_reduce
                nc.vector.tensor_add(
                    hiv[:, :, 0:32], e3[:, :, 0:32], e3[:, :, 32:64])
                off, w = 0, 32
                while w > 1:
                    nw = w // 2
                    noff = off + w
                    dst = (sume[:][:, :, None] if nw == 1
                           else hiv[:, :, noff:noff + nw])
                    nc.vector.tensor_add(
                        dst, hiv[:, :, off:off + nw],
                        hiv[:, :, off + nw:off + w])
                    off, w = noff, nw
                nc.vector.reciprocal(isum[:], sume[:])
                nc.vector.tensor_mul(scc[:], msk_sb[:], isum[:])

                # ---- phase 5: separable 3x3 box-sum on the packed
                # buffer, in two column chunks so chunk 0 starts while late
                # keep-T copies are still landing. kb2 = kb shifted by one
                # (4x-mode copy) so all h-pass taps are 4B-aligned (DVE 2x).
                # h3s[j] = kb[j] + kb[j+1] + kb[j+2]  (h[j+1], shifted)
                # cnt[c] = h[G1+c-192] + h[G1+c] + h[G1+c+192], h = h3s[j-1]
                HSPLIT = 1856
                VSPLIT = 1408
                for ci in range(2):
                    h0, h1 = (0, HSPLIT) if ci == 0 else (HSPLIT, KBW - 2)
                    c0, c1 = (0, VSPLIT) if ci == 0 else (VSPLIT, PKW)
                    nc.vector.tensor_copy(kb2[:, h0:h1], kb[:, h0 + 1:h1 + 1])
                    nc.vector.tensor_add(
                        h3s[:, h0:h1], kb[:, h0:h1], kb[:, h0 + 2:h1 + 2])
                    nc.vector.tensor_add(
                        h3s[:, h0:h1], h3s[:, h0:h1], kb2[:, h0:h1])
                    nc.vector.tensor_add(
                        cnt2[:, c0:c1], h3s[:, G1 - 193 + c0:G1 - 193 + c1],
                        h3s[:, G1 + 191 + c0:G1 + 191 + c1])
                    nc.vector.tensor_add(
                        cnt2[:, c0:c1], cnt2[:, c0:c1],
                        h3s[:, G1 - 1 + c0:G1 - 1 + c1])
                    if ci == 0:
                        # sustained re-warm burst riding the rest of the box
                        # phase so the VLAD tail starts at K=8/8
                        dh = pp.tile([128, 512], f32, tag="pm", bufs=2)
                        nc.tensor.matmul(
                            dh[0:64, 0:64], lhsT=h3s[:, 0:64],
                            rhs=h3s[:, 0:64], start=True, stop=True)
                        for _ in range(9):
                            dh = pp.tile([128, 512], f32, tag="pm", bufs=2)
                            nc.tensor.matmul(
                                dh[0:64, :], lhsT=warm[:, 0:64], rhs=warm[:],
                                start=True, stop=True)
                    else:
                        ham_keep(cnt2[:, c0:c0 + 64])

                # ---- phase 6: cnt-T back to [l, K], fuse w2 = cnt*scc*exp,
                # and immediately accumulate VLAD for each finished tile.
                # xnT arrives permuted in VLAD slot order (5 waves).
                x3 = xnt[:].rearrange("(a p) c -> p a c", p=128)
                xt3 = xnt_sb[:].rearrange("p (a c) -> p a c", c=C)
                nwav = (NT + XW - 1) // XW
                for wv in range(nwav):
                    n = min(XW, NT - wv * XW)
                    nc.sync.dma_start(
                        xt3[:, wv * XW:wv * XW + n, :],
                        x3[:, wv * XW:wv * XW + n, :],
                    )

                slot = 0
                started = [False, False]   # col group A (tiles<20), B

                def vlad_mm(tl, last):
                    nonlocal slot
                    grp = 0 if tl < 20 else 1
                    rows = slice(grp * 64, grp * 64 + 64)
                    lt = w2[:, tl * K:(tl + 1) * K]
                    if slot % XW == 0:     # absorb this wave's DMA wait
                        dw = pp.tile([128, 512], f32, tag="pm", bufs=2)
                        nc.tensor.matmul(
                            dw[0:64, 0:64],
                            lhsT=xnt_sb[:, slot * C:slot * C + 64],
                            rhs=xnt_sb[:, slot * C:slot * C + 64],
                            start=True, stop=True)
                    nc.tensor.matmul(
                        pv0[rows, :], lhsT=lt,
                        rhs=xnt_sb[:, slot * C:(slot + 1) * C],
                        start=not started[grp], stop=last,
                        tile_position=(0, grp * 64),
                        skip_group_check=True)
                    nc.tensor.matmul(
                        pv1[rows, 0:1], lhsT=lt, rhs=warm[:, 0:1],
                        start=not started[grp], stop=last,
                        tile_position=(0, grp * 64),
                        skip_group_check=True)
                    started[grp] = True
                    slot += 1

                def w2_fuse(tl, src, eng=None):
                    # alternate DVE / gpsimd so neither engine gates the tail
                    (eng or nc.vector).scalar_tensor_tensor(
                        w2[:, tl * K:(tl + 1) * K], src,
                        scc[:, tl:tl + 1], expb[:, tl * K:(tl + 1) * K],
                        op0=OP.mult, op1=OP.mult)

                # work items: 17 pair transposes then 5 singles; transposes
                # are emitted 3 ahead of their consumers (pt bufs=4) so the
                # PE never stalls on the act/DVE pipeline behind it
                def emit_T(i):
                    pc = pp.tile([128, 128], bf16, tag="pt", bufs=4)
                    if i < len(CNT_PAIRS):
                        j = CNT_PAIRS[i]
                        nc.tensor.transpose(
                            pc[:], cnt2[:, j * 128:(j + 1) * 128], id_sb[:])
                    else:
                        t = (CNT_SINGLE_A + CNT_SINGLE_B)[i - len(CNT_PAIRS)]
                        if t < 20:
                            nc.tensor.transpose(
                                pc[:, 0:K], cnt2[0:64, t * 128:(t + 1) * 128],
                                id_sb[0:64, 0:K])
                        else:
                            nc.tensor.transpose(
                                pc[:, 0:K],
                                cnt2[64:128,
                                     (t - POF) * 128:(t - POF + 1) * 128],
                                id_sb[64:128, 64:64 + K])
                    return pc

                def consume(i, pc):
                    if i < len(CNT_PAIRS):
                        j = CNT_PAIRS[i]
                        cl = cnt_lk[:, i * 128:(i + 1) * 128]
                        nc.scalar.copy(cl, pc[:])
                        tA, tB = j, j + POF
                        w2_fuse(tA, cl[:, 0:K])
                        w2_fuse(tB, cl[:, K:128])
                        vlad_mm(tA, last=False)
                        vlad_mm(tB, last=False)
                    else:
                        t = (CNT_SINGLE_A + CNT_SINGLE_B)[i - len(CNT_PAIRS)]
                        cl = cnt_lk[:, NPAIR * 128 + (i - len(CNT_PAIRS)) * K:
                                    NPAIR * 128 + (i - len(CNT_PAIRS) + 1) * K]
                        nc.scalar.copy(cl, pc[:, 0:K])
                        w2_fuse(t, cl)
                        vlad_mm(t, last=(t in (2, 38)))

                NW = len(CNT_PAIRS) + 5
                pend = []
                for i in range(min(3, NW)):
                    pend.append(emit_T(i))
                for i in range(NW):
                    consume(i, pend[i])
                    if i + 3 < NW:
                        pend.append(emit_T(i + 3))

                # ---- phase 7: write this core's [128, C+1] partial sums;
                # host sums col groups + cores, applies centroid subtraction
                # and the two L2 normalizations
                nc.scalar.copy(vl_sb[:, 0:C], pv0[:])
                nc.scalar.copy(vl_sb[:, C:C + 1], pv1[:, 0:1])
                nc.sync.dma_start(y[:], vl_sb[:])
    _prune_waits(nc)
    return nc


def _prune_waits(nc):
    """Drop semaphore waits that are transitively implied by another wait on
    the same instruction (walrus codegen allows one hw wait per compute
    instruction; extra waits cost separate EVENT_SEMAPHORE instructions)."""
    insts = [ins for bb in nc.main_func.blocks for ins in bb.instructions]
    proc_events = {}
    waits_of = {}
    pending = {}    # engine -> waits of non-ticking instrs (e.g. Ldweights),
    #                 folded into the next ticking instr on that engine so the
    #                 transitive closure can see them (engines run in-order)
    for ins in insts:
        si = getattr(ins, "sync_info", None)
        if si is None:
            continue
        eng = getattr(ins, "engine", None)
        ow = [(w.ant_name, w.wait_value) for w in (si.on_wait or [])]
        carried = pending.get(eng, [])
        all_waits = carried + ow
        ticked = False
        for u in (si.on_update or []):
            if getattr(u, "update_mode", None) not in ("sem-inc", "sem-add-imm"):
                continue
            ticked = True
            lst = proc_events.setdefault(u.ant_name, [])
            prev = lst[-1][0] if lst else 0
            lst.append((prev + (u.update_value or 1), ins))
        waits_of[id(ins)] = all_waits if ticked else ow
        pending[eng] = [] if ticked else all_waits

    import bisect

    def prefix_index(sem, v):
        lst = proc_events.get(sem)
        if not lst:
            return None
        ticks = [t for t, _ in lst]
        i = bisect.bisect_left(ticks, v)
        return i if i < len(lst) else None

    memo = {}

    def holds(sem, v, depth=0):
        if depth > 6:
            return {}
        i = prefix_index(sem, v)
        if i is None:
            return {}
        key = (sem, i)
        if key in memo:
            return memo[key]
        memo[key] = {}
        out = {}
        inorder = not sem.startswith("Pool")
        rng = range(i + 1) if inorder else (i,)
        for j in rng:
            _, ins = proc_events[sem][j]
            for (s2, v2) in waits_of.get(id(ins), []):
                if out.get(s2, 0) < v2:
                    out[s2] = v2
                sub = holds(s2, v2, depth + 1)
                for s3, v3 in sub.items():
                    if out.get(s3, 0) < v3:
                        out[s3] = v3
        memo[key] = out
        return out

    own_tick = {}
    for sem, lst in proc_events.items():
        for tick, ins in lst:
            own_tick[(id(ins), sem)] = tick

    pruned = 0
    for ins in insts:
        si = getattr(ins, "sync_info", None)
        if si is None or not si.on_wait or len(si.on_wait) < 2:
            continue
        ow = list(si.on_wait)
        kept = list(ow)
        tn = type(ins).__name__
        is_dma = "DMA" in tn or "Drain" in tn
        for w in ow:
            if len(kept) == 1:
                break
            # same-queue FIFO rule, DMA instructions only: waiting on earlier
            # completions of the queue this DMA executes on is vacuous
            # (per-queue serial execution). Compute engines keep such waits:
            # the race detector requires them when APs overlap.
            if is_dma:
                mine = own_tick.get((id(ins), w.ant_name))
                if mine is not None and w.wait_value <= mine - 1:
                    kept.remove(w)
                    pruned += 1
                    continue
            others = [o for o in kept if o is not w]
            for o in others:
                h = holds(o.ant_name, o.wait_value)
                if h.get(w.ant_name, 0) >= w.wait_value:
                    kept.remove(w)
                    pruned += 1
                    break
        si.on_wait = kept
    return pruned


def _host_prep(x, conv_w, centroids):
    from concourse import mybir
    bf16np = mybir.dt.np(mybir.dt.bfloat16)
    fp8np = mybir.dt.np(mybir.dt.float8e4)

    x = np.ascontiguousarray(x, dtype=np.float32)
    norm = np.sqrt((x.astype(np.float64) ** 2).sum(0))
    xn = (x / np.maximum(norm, 1e-12)).astype(np.float32)    # [C,H,W]
    ii = np.arange(H, dtype=np.float64)
    mi = np.minimum(H - 1 - ii, ii)
    m = np.minimum(mi[:, None], mi[None, :])
    m4 = m ** 4
    # rescale so w2 = msk*soft*cnt fits fp8e4m3 range; the global scale
    # cancels in the intra-cluster L2 normalization on the host
    msk_full = (m4 / m4.max()).astype(np.float32)            # [H,W]

    xn_pad = np.zeros((C, H + 2, W), np.float32)
    xn_pad[:, 1:H + 1] = xn
    msk_pad = np.zeros((H + 2, W), np.float32)
    msk_pad[1:H + 1] = msk_full

    # packed small inputs
    cwtb = conv_w.T.astype(np.float32).reshape(CT, 128, K)
    cwtb = np.ascontiguousarray(cwtb.transpose(1, 0, 2)).reshape(128, CT * K)
    small8 = np.zeros((128, CT * K + 8), np.float32)
    small8[:, 0:CT * K] = cwtb
    small8[:, CT * K:] = 1.0
    small8 = small8.astype(fp8np)
    identb = np.eye(128, dtype=np.float32)
    mstack = np.concatenate([np.eye(K), np.eye(K)], 0).astype(np.float32)
    slot = np.array(SLOT_TILES)

    in_maps = []
    for core in range(M):
        r0 = core * RPC
        slab = np.ascontiguousarray(
            xn_pad[:, r0:r0 + RPC + 2, :].reshape(C, Ls))
        mskc = msk_pad[r0:r0 + RPC + 2].reshape(Ls).copy()
        mskc[0:W] = 0.0
        mskc[(RPC + 1) * W:] = 0.0                           # halo rows -> 0
        xnT = np.ascontiguousarray(slab.T).astype(bf16np)    # [Ls, C]
        # permute l-tiles into VLAD slot order
        xnT_perm = np.ascontiguousarray(
            xnT.reshape(NT, 128, C)[slot].reshape(Ls, C))
        smallb = np.zeros((128, 128 + K + NT), np.float32)
        smallb[:, 0:128] = identb
        smallb[:, 128:128 + K] = mstack
        smallb[:, 128 + K:] = mskc.reshape(NT, 128).T
        in_maps.append({
            "xnb": slab.astype(fp8np),
            "xnt": xnT_perm,
            "smallb": smallb.astype(bf16np),
            "small8": small8,
        })
    return in_maps


def _ensure_ntff_hook():
    """Install the axon NTFF profile hook if the image's antenv lacks it."""
    import types
    try:
        from antenv.axon_hooks import get_axon_ntff_profile_hook  # noqa: F401
        return
    except ImportError:
        pass
    if "/root/.axon_site" not in sys.path:
        sys.path.insert(0, "/root/.axon_site")
    from trn_agent_boot.trn_boot import _ntff_profile_via_ctypes
    hook = _ntff_profile_via_ctypes("/opt/axon/libaxon_pjrt.so")
    mod = types.ModuleType("antenv.axon_hooks")
    mod.get_axon_ntff_profile_hook = lambda: hook
    mod.set_axon_ntff_profile_hook = lambda h: None
    import antenv
    antenv.axon_hooks = mod
    sys.modules["antenv.axon_hooks"] = mod


def _install_neff_cache():
    """Cache compiled NEFFs across processes, keyed by BIR content hash."""
    import hashlib
    import shutil
    import concourse.bass2jax as b2j

    orig = b2j.compile_bir_kernel
    if getattr(orig, "_neff_cached", False):
        return

    def cached(bir_json, tmpdir, neff_name="file.neff"):
        h = hashlib.sha256(
            bir_json if isinstance(bir_json, bytes) else bir_json.encode()
        ).hexdigest()[:24]
        cdir = "/tmp/neff_cache"
        os.makedirs(cdir, exist_ok=True)
        cpath = os.path.join(cdir, h + ".neff")
        if os.path.exists(cpath):
            dst = os.path.join(tmpdir, neff_name)
            os.makedirs(tmpdir, exist_ok=True)
            shutil.copy(cpath, dst)
            return dst
        out = orig(bir_json, tmpdir, neff_name=neff_name)
        shutil.copy(out, cpath)
        return out

    cached._neff_cached = True
    b2j.compile_bir_kernel = cached


def kernel(x, conv_w, centroids):
    import concourse.bass_utils as bu
    from concourse.bass_utils import run_bass_kernel_spmd
    _install_neff_cache()
    if TRACE:
        _ensure_ntff_hook()
        bu.upload_artifacts = lambda tmpdir: "local://" + tmpdir

    if "nc" not in _CACHE:
        _CACHE["nc"] = _build_nc()
    nc = _CACHE["nc"]
    in_maps = _host_prep(np.asarray(x), np.asarray(conv_w), np.asarray(centroids))
    res = run_bass_kernel_spmd(nc, in_maps, list(range(M)), trace=TRACE)
    _CACHE["last"] = res
    red = np.zeros((128, C + 1), np.float64)
    for r in res.results:
        red += np.asarray(r["y"], dtype=np.float64)
    redk = red[0:64] + red[64:128]                           # [K, C+1]
    vlad = redk[:, :C] - redk[:, C:C + 1] * np.asarray(centroids, np.float64)
    vlad /= np.maximum(np.sqrt((vlad ** 2).sum(1))[:, None], 1e-12)
    v = vlad.reshape(1, K * C)
    v /= np.maximum(np.sqrt((v ** 2).sum()), 1e-12)
    return v.astype(np.float32)


# revision 60
# speedup vs baseline: 1.1768x; 1.0680x over previous
"""NetVLAD (vq_codebook) Trainium2 Bass kernel, 8-way spatially sharded.

Math (validated in numpy + CoreSim to rel ~2e-3 vs reference, gate 2e-2):
  xn = x / ||x||_C per location (host); logits = conv_w @ xn; soft = softmax_K
  fold(unfold(soft) * top2keep) == soft * cnt, cnt = 3x3 box-sum of the
  per-cluster top-2 indicator. vlad = sa2 @ xn.T - rowsum(sa2) * centroids,
  then intra + global L2 norm (host).

vs the fp32 baseline (174us -> ~63us HW):
- fp8e4m3 x in both layouts + conv weights + w2 (mask host-rescaled into
  fp8 range; the global scale cancels in the final L2 norms); bf16
  elsewhere; fp32 PSUM accumulation.
- col-tiled K=64 matmuls: two concurrent 64-col groups in the 128x128 PE
  array (logits c-tile pairs, VLAD l-tile pairs; host sums the halves).
- logits transposed+halves-summed by a regular matmul against a stacked
  identity; exp batched 8 tiles per PSUM bank.
- top-2 via a tournament (hi/lo chains) in DVE 2x mode, two L stripes so
  stripe A's keep-transposes overlap stripe B's chain.
- 3x3 box-sum on a 128-partition packed buffer (two L halves stacked in
  the partition dim via paired keep-transposes), separable h/v passes,
  shifted kb copy for 4B alignment.
- sustained PE warm-up bursts (HAM K=8/8) over the DMA-bound start and
  the box phase; packed small inputs; stream-first DMA order.

Sharding: H=192 rows split 8 ways (24 rows/core + 1 halo row each side).
[128, C+1] partial VLAD sums summed on host across col groups and cores.
"""
import os
import sys

sys.path.insert(0, "/opt/trn_rl_repo")
os.environ.setdefault("MYCRO_LOCAL_CACHE", "1")

import numpy as np

C, H, W, K = 512, 192, 192, 64
M = 8                      # cores
RPC = H // M               # 24 rows per core
Ls = (RPC + 2) * W         # 4992 slab locations (incl. 1 halo row each side)
NT = Ls // 128             # 39 l-tiles
CT = C // 128              # 4 c-tiles
G1 = 257                   # kb guard (odd -> v-pass offsets even for DVE 2x)
PKW = 22 * 128             # 2816 packed box-sum columns (22 tiles)
KBW = G1 + PKW + G1        # 3330 keep-buffer width
POF = 17                   # B row-group holds tiles 17..38 (l offset 2176)
NPAIR = 22                 # keep-T pair transposes (t, t+17), t=0..21
XW = 8                     # xnt DMA batching (tiles per DMA wave)

# cnt-T schedule: pairs (j, j+17) j=3..19 (A rows->tile j, B rows->tile j+17),
# then singles: tiles 0..2 from A rows, tiles 37..38 from B rows.
CNT_PAIRS = list(range(3, 20))
CNT_SINGLE_A = [0, 1, 2]
CNT_SINGLE_B = [37, 38]
# VLAD slot order = w2 availability order; host permutes xnT rows to match.
SLOT_TILES = []
for _j in CNT_PAIRS:
    SLOT_TILES += [_j, _j + POF]
SLOT_TILES += CNT_SINGLE_A + CNT_SINGLE_B        # 39 slots

TRACE = False              # set by test.py for profiling runs
_CACHE = {}


def _build_nc():
    import concourse.bass as bass
    import concourse.tile as tile
    from concourse import mybir

    f32 = mybir.dt.float32
    bf16 = mybir.dt.bfloat16
    AF = mybir.ActivationFunctionType
    OP = mybir.AluOpType
    AX = mybir.AxisListType

    fp8 = mybir.dt.float8e4
    nc = bass.Bass()
    xnb = nc.dram_tensor("xnb", [C, Ls], fp8, kind="ExternalInput")
    xnt = nc.dram_tensor("xnt", [Ls, C], bf16, kind="ExternalInput")
    # small inputs packed into two tensors (fewer serialized DMA triggers):
    # smallb = identb(128) | mstack(64) | msk-as-bf16(NT) cols, bf16
    # small8 = cwt(4*64) | ones8(8) cols, fp8
    smallb = nc.dram_tensor("smallb", [128, 128 + K + NT], bf16,
                            kind="ExternalInput")
    small8 = nc.dram_tensor("small8", [128, CT * K + 8], fp8,
                            kind="ExternalInput")
    y = nc.dram_tensor("y", [128, C + 1], f32, kind="ExternalOutput")

    with tile.TileContext(nc) as tc:
        with tc.tile_pool(name="big", bufs=1) as big:
            # persistent SBUF tensors
            xnb_sb = big.tile([128, CT * Ls], fp8, tag="xnb")
            xnt_sb = big.tile([128, NT * C], bf16, tag="xnt")
            logklb = big.tile([128, Ls], bf16, tag="logklb")
            expb = big.tile([128, NT * K], bf16, tag="expb")
            tmpb = big.tile([128, NT * K], bf16, tag="tmpb")
            keep2 = big.tile([128, NPAIR * 128], bf16, tag="keep2")
            kb = big.tile([128, KBW], bf16, tag="kb")
            kb2 = big.tile([128, KBW], bf16, tag="kb2")
            h3s = big.tile([128, KBW], bf16, tag="h3s")
            cnt2 = big.tile([128, PKW], bf16, tag="cnt2")
            cnt_lk = big.tile([128, NPAIR * 128 + 5 * K], bf16, tag="cntlk")
            w2 = big.tile([128, NT * K], bf16, tag="w2")
            trs = big.tile([128, NT * K], bf16, tag="trs")
            trs2 = big.tile([128, NT * 32], bf16, tag="trs2")
            smallb_sb = big.tile([128, 128 + K + NT], bf16, tag="smallb")
            small8_sb = big.tile([128, CT * K + 8], fp8, tag="small8")
            id_sb = smallb_sb[:, 0:128]
            mst_sb = smallb_sb[:, 128:128 + K]
            msk_sb = smallb_sb[:, 128 + K:128 + K + NT]
            cwt_sb = small8_sb[:, 0:CT * K]
            ones8_sb = small8_sb[:, CT * K:CT * K + 8]
            warm = big.tile([128, 512], bf16, tag="warm")
            sume = big.tile([128, NT], f32, tag="sume")
            m1b = big.tile([128, NT], f32, tag="m1b")
            m2b = big.tile([128, NT], f32, tag="m2b")
            isum = big.tile([128, NT], f32, tag="isum")
            scc = big.tile([128, NT], f32, tag="scc")
            vl_sb = big.tile([128, C + 1], f32, tag="vl")
            scr = big.tile([128, 4], f32, tag="scr")

            # zero the keep-buffer guards (both row groups); build the PE
            # warm-up operand without any DMA dependency
            nc.vector.memset(kb[:, 0:G1], 0.0)
            nc.vector.memset(kb[:, G1 + PKW:KBW], 0.0)
            nc.vector.memset(warm[:], 1.0)

            # DMA order: first xnb chunk first (it gates the logits), then
            # the two packed small inputs, then the remaining chunks
            xc3 = xnb[:].rearrange("(ct p) l -> p ct l", p=128)
            xs3 = xnb_sb[:].rearrange("p (ct l) -> p ct l", l=Ls)
            DCH = 6
            csz = Ls // DCH              # 832 cols per chunk, all c-tiles
            nc.sync.dma_start(xs3[:, :, 0:csz], xc3[:, :, 0:csz])
            nc.sync.dma_start(smallb_sb[:], smallb[:])
            nc.sync.dma_start(small8_sb[:], small8[:])
            for j in range(1, DCH):
                nc.sync.dma_start(
                    xs3[:, :, j * csz:(j + 1) * csz],
                    xc3[:, :, j * csz:(j + 1) * csz],
                )
            # touch ops absorbing the packed-small-DMA completions so
            # downstream compute carries at most one sync wait each
            nc.scalar.copy(scr[:, 0:1], msk_sb[:, 0:1])
            nc.vector.tensor_copy(scr[:, 1:2], msk_sb[:, 1:2])

            with tc.tile_pool(name="pp", bufs=1, space="PSUM") as pp:
                pv0 = pp.tile([128, C], f32, tag="pv0", bufs=1)
                pv1 = pp.tile([128, 8], f32, tag="pv1", bufs=1)
                # sustained warm-up burst (~4us of PE activity, no DMA deps):
                # trips the HAM to K=8/8 during the DMA-bound startup so the
                # logits matmuls run at 2.4 GHz from the first block
                for i in range(10):
                    dummy = pp.tile([128, 512], f32, tag="pm", bufs=2)
                    nc.tensor.matmul(dummy[0:64, :], lhsT=warm[:, 0:64],
                                     rhs=warm[:], start=True, stop=True)
                # absorb the packed small-input DMA sems on the PE stream
                dummy = pp.tile([128, 512], f32, tag="pm", bufs=2)
                nc.tensor.matmul(dummy[0:64, 0:64], lhsT=cwt_sb[:, 0:64],
                                 rhs=cwt_sb[:, 0:64], start=True, stop=True)
                nc.tensor.matmul(dummy[0:64, 0:64], lhsT=id_sb[:, 0:64],
                                 rhs=mst_sb[:, 0:64], start=True, stop=True)

                # ---- phase 1: logits [K, L] col-tiled (ct0/ct2 -> psum rows
                # 0:64, ct1/ct3 -> rows 64:128), streamed over xnb chunks
                nblk = (Ls + 511) // 512
                touched = set()
                for b in range(nblk):
                    w = min(512, Ls - b * 512)
                    for j in range((b * 512) // csz,
                                   (b * 512 + w - 1) // csz + 1):
                        if j not in touched:
                            touched.add(j)
                            dj = pp.tile([128, 512], f32, tag="pm", bufs=2)
                            nc.tensor.matmul(
                                dj[0:64, 0:64],
                                lhsT=xnb_sb[:, j * csz:j * csz + 64],
                                rhs=xnb_sb[:, j * csz:j * csz + 64],
                                start=True, stop=True)
                    plog = pp.tile([128, 512], f32, tag="pm", bufs=2)
                    for ct in range(CT):
                        nc.tensor.matmul(
                            plog[(ct % 2) * 64:(ct % 2) * 64 + 64, 0:w],
                            lhsT=cwt_sb[:, ct * K:(ct + 1) * K],
                            rhs=xnb_sb[:, ct * Ls + b * 512:
                                       ct * Ls + b * 512 + w],
                            start=(ct < 2),
                            stop=(ct >= 2),
                            tile_position=(0, (ct % 2) * 64),
                            skip_group_check=True,
                        )
                    nc.scalar.copy(logklb[:, b * 512:b * 512 + w],
                                   plog[:, 0:w])

                # ---- phase 2: transpose logits tiles with Mstack (sums the
                # two col-group partials) then exp; 8 tiles batched per PSUM
                # bank so one activation covers 8 tiles
                t = 0
                while t < NT:
                    n = min(8, NT - t)
                    pt = pp.tile([128, 8 * K], f32, tag="pt", bufs=4)
                    for i in range(n):
                        # regular matmul: out = logklb_tile.T @ Mstack sums
                        # the two col-group partials while transposing
                        nc.tensor.matmul(
                            pt[:, i * K:(i + 1) * K],
                            lhsT=logklb[:, (t + i) * 128:(t + i + 1) * 128],
                            rhs=mst_sb[:], start=True, stop=True,
                            skip_group_check=True)
                    nc.scalar.activation(
                        expb[:, t * K:(t + n) * K], pt[:, 0:n * K], AF.Exp)
                    t += n

                # ---- phase 3: batched top-2 + softmax denominators
                e3 = expb[:].rearrange("p (t k) -> p t k", k=K)
                m2bc = m2b[:][:, :, None].broadcast_to([128, NT, K])
                k4 = keep2[:].rearrange("p (t k) -> p t k", k=128)
                hiv = tmpb[:].rearrange("p (t k) -> p t k", k=K)
                lov = trs[:].rearrange("p (t k) -> p t k", k=K)
                tt2 = trs2[:].rearrange("p (t k) -> p t k", k=32)

                def ham_keep(src):
                    # small matmul reading `src`: keeps the PE HAM warm during
                    # the DVE-heavy phases and absorbs that engine's sem on
                    # the PE stream (later PE waits become prunable)
                    n = min(src.shape[-1], 64)
                    dh = pp.tile([128, 512], f32, tag="pm", bufs=2)
                    nc.tensor.matmul(dh[0:n, 0:n], lhsT=src[:, 0:n],
                                     rhs=src[:, 0:n], start=True, stop=True)

                def top2_stripe(ts, te):
                    # tournament second-max: hi = block max, lo = block
                    # second-max; merging two blocks:
                    #   hi' = max(hi_l, hi_r)
                    #   lo' = max(min(hi_l, hi_r), max(lo_l, lo_r))
                    # all stages run in DVE 2x mode (bf16, aligned halves)
                    nc.vector.tensor_tensor(
                        hiv[:, ts:te, 0:32], e3[:, ts:te, 0:32],
                        e3[:, ts:te, 32:64], op=OP.max)
                    nc.vector.tensor_tensor(
                        lov[:, ts:te, 0:32], e3[:, ts:te, 0:32],
                        e3[:, ts:te, 32:64], op=OP.min)
                    off, w = 0, 32
                    while w > 1:
                        nw = w // 2
                        noff = off + w
                        hl = hiv[:, ts:te, off:off + nw]
                        hr = hiv[:, ts:te, off + nw:off + w]
                        ll = lov[:, ts:te, off:off + nw]
                        lr = lov[:, ts:te, off + nw:off + w]
                        t1 = tt2[:, ts:te, 0:nw]
                        t2 = tt2[:, ts:te, nw:2 * nw]
                        nc.vector.tensor_tensor(t1, hl, hr, op=OP.min)
                        nc.vector.tensor_tensor(t2, ll, lr, op=OP.max)
                        if nw == 1:
                            nc.vector.tensor_tensor(
                                m2b[:, ts:te][:, :, None], t1, t2, op=OP.max)
                        else:
                            nc.vector.tensor_tensor(
                                lov[:, ts:te, noff:noff + nw], t1, t2,
                                op=OP.max)
                            nc.vector.tensor_tensor(
                                hiv[:, ts:te, noff:noff + nw], hl, hr,
                                op=OP.max)
                        off, w = noff, nw

                # two L stripes so stripe A's keep-T/kb work overlaps
                # stripe B's top-2 chain (stripe A = tiles 0..21, B = 22..38)
                for (ts, te) in ((0, NPAIR), (NPAIR, NT)):
                    top2_stripe(ts, te)
                    ham_keep(tmpb[:, 0:32])
                    if ts == 0:
                        # keep pair-slot cols 0:64 = tiles 0..21 (stripe A)
                        # and cols 64:128 of slots 0..4 = tiles 17..21
                        nc.vector.tensor_tensor(
                            k4[:, :, 0:K], e3[:, 0:NPAIR], m2bc[:, 0:NPAIR],
                            op=OP.is_ge)
                        nc.vector.tensor_tensor(
                            k4[:, 0:5, K:128], e3[:, POF:POF + 5],
                            m2bc[:, POF:POF + 5], op=OP.is_ge)
                    else:
                        # cols 64:128 of slots 5..21 = tiles 22..38 (B)
                        nc.vector.tensor_tensor(
                            k4[:, 5:NPAIR, K:128], e3[:, POF + 5:NT],
                            m2bc[:, POF + 5:NT], op=OP.is_ge)

                # ---- phase 4: keep-T pair transposes into the packed
                # guarded buffer: kb rows 0:64 = keep[K, l] for l tiles 0..21,
                # rows 64:128 = tiles 17..38, same columns
                for t in range(NPAIR):
                    pk = pp.tile([128, 128], bf16, tag="pt", bufs=4)
                    nc.tensor.transpose(
                        pk[:], keep2[:, t * 128:(t + 1) * 128], id_sb[:])
                    nc.scalar.copy(kb[:, G1 + t * 128:G1 + (t + 1) * 128],
                                   pk[:])
                    if t % 6 == 5:   # keep the PE HAM warm through this phase
                        dh = pp.tile([128, 512], f32, tag="pm", bufs=2)
                        nc.tensor.matmul(
                            dh[0:64, 0:64], lhsT=warm[:, 0:64],
                            rhs=warm[:, 0:64], start=True, stop=True)

                # softmax denominators off the critical chain (run on DVE
                # while the PE does keep-T transposes); halving add-tree in
                # 2x mode beats the all-1x tensor_reduce
                nc.vector.tensor_add(
                    hiv[:, :, 0:32], e3[:, :, 0:32], e3[:, :, 32:64])
                off, w = 0, 32
                while w > 1:
                    nw = w // 2
                    noff = off + w
                    dst = (sume[:][:, :, None] if nw == 1
                           else hiv[:, :, noff:noff + nw])
                    nc.vector.tensor_add(
                        dst, hiv[:, :, off:off + nw],
                        hiv[:, :, off + nw:off + w])
                    off, w = noff, nw
                nc.vector.reciprocal(isum[:], sume[:])
                nc.vector.tensor_mul(scc[:], msk_sb[:], isum[:])

                # ---- phase 5: separable 3x3 box-sum on the packed
                # buffer, in two column chunks so chunk 0 starts while late
                # keep-T copies are still landing. kb2 = kb shifted by one
                # (4x-mode copy) so all h-pass taps are 4B-aligned (DVE 2x).
                # h3s[j] = kb[j] + kb[j+1] + kb[j+2]  (h[j+1], shifted)
                # cnt[c] = h[G1+c-192] + h[G1+c] + h[G1+c+192], h = h3s[j-1]
                HSPLIT = 1856
                VSPLIT = 1408
                for ci in range(2):
                    h0, h1 = (0, HSPLIT) if ci == 0 else (HSPLIT, KBW - 2)
                    c0, c1 = (0, VSPLIT) if ci == 0 else (VSPLIT, PKW)
                    nc.vector.tensor_copy(kb2[:, h0:h1], kb[:, h0 + 1:h1 + 1])
                    nc.vector.tensor_add(
                        h3s[:, h0:h1], kb[:, h0:h1], kb[:, h0 + 2:h1 + 2])
                    nc.vector.tensor_add(
                        h3s[:, h0:h1], h3s[:, h0:h1], kb2[:, h0:h1])
                    nc.vector.tensor_add(
                        cnt2[:, c0:c1], h3s[:, G1 - 193 + c0:G1 - 193 + c1],
                        h3s[:, G1 + 191 + c0:G1 + 191 + c1])
                    nc.vector.tensor_add(
                        cnt2[:, c0:c1], cnt2[:, c0:c1],
                        h3s[:, G1 - 1 + c0:G1 - 1 + c1])
                    if ci == 0:
                        # sustained re-warm burst riding the rest of the box
                        # phase so the VLAD tail starts at K=8/8
                        dh = pp.tile([128, 512], f32, tag="pm", bufs=2)
                        nc.tensor.matmul(
                            dh[0:64, 0:64], lhsT=h3s[:, 0:64],
                            rhs=h3s[:, 0:64], start=True, stop=True)
                        for _ in range(9):
                            dh = pp.tile([128, 512], f32, tag="pm", bufs=2)
                            nc.tensor.matmul(
                                dh[0:64, :], lhsT=warm[:, 0:64], rhs=warm[:],
                                start=True, stop=True)
                    else:
                        ham_keep(cnt2[:, c0:c0 + 64])

                # ---- phase 6: cnt-T back to [l, K], fuse w2 = cnt*scc*exp,
                # and immediately accumulate VLAD for each finished tile.
                # xnT arrives permuted in VLAD slot order (5 waves).
                x3 = xnt[:].rearrange("(a p) c -> p a c", p=128)
                xt3 = xnt_sb[:].rearrange("p (a c) -> p a c", c=C)
                nwav = (NT + XW - 1) // XW
                for wv in range(nwav):
                    n = min(XW, NT - wv * XW)
                    nc.sync.dma_start(
                        xt3[:, wv * XW:wv * XW + n, :],
                        x3[:, wv * XW:wv * XW + n, :],
                    )

                slot = 0
                started = [False, False]   # col group A (tiles<20), B

                def vlad_mm(tl, last):
                    nonlocal slot
                    grp = 0 if tl < 20 else 1
                    rows = slice(grp * 64, grp * 64 + 64)
                    lt = w2[:, tl * K:(tl + 1) * K]
                    if slot % XW == 0:     # absorb this wave's DMA wait
                        dw = pp.tile([128, 512], f32, tag="pm", bufs=2)
                        nc.tensor.matmul(
                            dw[0:64, 0:64],
                            lhsT=xnt_sb[:, slot * C:slot * C + 64],
                            rhs=xnt_sb[:, slot * C:slot * C + 64],
                            start=True, stop=True)
                    nc.tensor.matmul(
                        pv0[rows, :], lhsT=lt,
                        rhs=xnt_sb[:, slot * C:(slot + 1) * C],
                        start=not started[grp], stop=last,
                        tile_position=(0, grp * 64),
                        skip_group_check=True)
                    nc.tensor.matmul(
                        pv1[rows, 0:1], lhsT=lt, rhs=warm[:, 0:1],
                        start=not started[grp], stop=last,
                        tile_position=(0, grp * 64),
                        skip_group_check=True)
                    started[grp] = True
                    slot += 1

                def w2_fuse(tl, src, eng=None):
                    # alternate DVE / gpsimd so neither engine gates the tail
                    (eng or nc.vector).scalar_tensor_tensor(
                        w2[:, tl * K:(tl + 1) * K], src,
                        scc[:, tl:tl + 1], expb[:, tl * K:(tl + 1) * K],
                        op0=OP.mult, op1=OP.mult)

                # work items: 17 pair transposes then 5 singles; transposes
                # are emitted 3 ahead of their consumers (pt bufs=4) so the
                # PE never stalls on the act/DVE pipeline behind it
                def emit_T(i):
                    pc = pp.tile([128, 128], bf16, tag="pt", bufs=4)
                    if i < len(CNT_PAIRS):
                        j = CNT_PAIRS[i]
                        nc.tensor.transpose(
                            pc[:], cnt2[:, j * 128:(j + 1) * 128], id_sb[:])
                    else:
                        t = (CNT_SINGLE_A + CNT_SINGLE_B)[i - len(CNT_PAIRS)]
                        if t < 20:
                            nc.tensor.transpose(
                                pc[:, 0:K], cnt2[0:64, t * 128:(t + 1) * 128],
                                id_sb[0:64, 0:K])
                        else:
                            nc.tensor.transpose(
                                pc[:, 0:K],
                                cnt2[64:128,
                                     (t - POF) * 128:(t - POF + 1) * 128],
                                id_sb[64:128, 64:64 + K])
                    return pc

                def consume(i, pc):
                    if i < len(CNT_PAIRS):
                        j = CNT_PAIRS[i]
                        cl = cnt_lk[:, i * 128:(i + 1) * 128]
                        nc.scalar.copy(cl, pc[:])
                        tA, tB = j, j + POF
                        w2_fuse(tA, cl[:, 0:K])
                        w2_fuse(tB, cl[:, K:128])
                        vlad_mm(tA, last=False)
                        vlad_mm(tB, last=False)
                    else:
                        t = (CNT_SINGLE_A + CNT_SINGLE_B)[i - len(CNT_PAIRS)]
                        cl = cnt_lk[:, NPAIR * 128 + (i - len(CNT_PAIRS)) * K:
                                    NPAIR * 128 + (i - len(CNT_PAIRS) + 1) * K]
                        nc.scalar.copy(cl, pc[:, 0:K])
                        w2_fuse(t, cl)
                        vlad_mm(t, last=(t in (2, 38)))

                NW = len(CNT_PAIRS) + 5
                pend = []
                for i in range(min(3, NW)):
                    pend.append(emit_T(i))
                for i in range(NW):
                    consume(i, pend[i])
                    if i + 3 < NW:
                        pend.append(emit_T(i + 3))

                # ---- phase 7: write this core's [128, C+1] partial sums;
                # host sums col groups + cores, applies centroid subtraction
                # and the two L2 normalizations
                nc.scalar.copy(vl_sb[:, 0:C], pv0[:])
                nc.scalar.copy(vl_sb[:, C:C + 1], pv1[:, 0:1])
                nc.sync.dma_start(y[:], vl_sb[:])
    _prune_waits(nc)
    return nc


def _prune_waits(nc):
    """Drop semaphore waits that are transitively implied by another wait on
    the same instruction (walrus codegen allows one hw wait per compute
    instruction; extra waits cost separate EVENT_SEMAPHORE instructions)."""
    insts = [ins for bb in nc.main_func.blocks for ins in bb.instructions]
    proc_events = {}
    waits_of = {}
    pending = {}    # engine -> waits of non-ticking instrs (e.g. Ldweights),
    #                 folded into the next ticking instr on that engine so the
    #                 transitive closure can see them (engines run in-order)
    for ins in insts:
        si = getattr(ins, "sync_info", None)
        if si is None:
            continue
        eng = getattr(ins, "engine", None)
        ow = [(w.ant_name, w.wait_value) for w in (si.on_wait or [])]
        carried = pending.get(eng, [])
        all_waits = carried + ow
        ticked = False
        for u in (si.on_update or []):
            if getattr(u, "update_mode", None) not in ("sem-inc", "sem-add-imm"):
                continue
            ticked = True
            lst = proc_events.setdefault(u.ant_name, [])
            prev = lst[-1][0] if lst else 0
            lst.append((prev + (u.update_value or 1), ins))
        waits_of[id(ins)] = all_waits if ticked else ow
        pending[eng] = [] if ticked else all_waits

    import bisect

    def prefix_index(sem, v):
        lst = proc_events.get(sem)
        if not lst:
            return None
        ticks = [t for t, _ in lst]
        i = bisect.bisect_left(ticks, v)
        return i if i < len(lst) else None

    memo = {}

    def holds(sem, v, depth=0):
        if depth > 6:
            return {}
        i = prefix_index(sem, v)
        if i is None:
            return {}
        key = (sem, i)
        if key in memo:
            return memo[key]
        memo[key] = {}
        out = {}
        inorder = not sem.startswith("Pool")
        rng = range(i + 1) if inorder else (i,)
        for j in rng:
            _, ins = proc_events[sem][j]
            for (s2, v2) in waits_of.get(id(ins), []):
                if out.get(s2, 0) < v2:
                    out[s2] = v2
                sub = holds(s2, v2, depth + 1)
                for s3, v3 in sub.items():
                    if out.get(s3, 0) < v3:
                        out[s3] = v3
        memo[key] = out
        return out

    own_tick = {}
    for sem, lst in proc_events.items():
        for tick, ins in lst:
            own_tick[(id(ins), sem)] = tick

    pruned = 0
    for ins in insts:
        si = getattr(ins, "sync_info", None)
        if si is None or not si.on_wait or len(si.on_wait) < 2:
            continue
        ow = list(si.on_wait)
        kept = list(ow)
        tn = type(ins).__name__
        is_dma = "DMA" in tn or "Drain" in tn
        for w in ow:
            if len(kept) == 1:
                break
            # same-queue FIFO rule, DMA instructions only: waiting on earlier
            # completions of the queue this DMA executes on is vacuous
            # (per-queue serial execution). Compute engines keep such waits:
            # the race detector requires them when APs overlap.
            if is_dma:
                mine = own_tick.get((id(ins), w.ant_name))
                if mine is not None and w.wait_value <= mine - 1:
                    kept.remove(w)
                    pruned += 1
                    continue
            others = [o for o in kept if o is not w]
            for o in others:
                h = holds(o.ant_name, o.wait_value)
                if h.get(w.ant_name, 0) >= w.wait_value:
                    kept.remove(w)
                    pruned += 1
                    break
        si.on_wait = kept
    return pruned


def _host_prep(x, conv_w, centroids):
    from concourse import mybir
    bf16np = mybir.dt.np(mybir.dt.bfloat16)
    fp8np = mybir.dt.np(mybir.dt.float8e4)

    x = np.ascontiguousarray(x, dtype=np.float32)
    norm = np.sqrt((x.astype(np.float64) ** 2).sum(0))
    xn = (x / np.maximum(norm, 1e-12)).astype(np.float32)    # [C,H,W]
    ii = np.arange(H, dtype=np.float64)
    mi = np.minimum(H - 1 - ii, ii)
    m = np.minimum(mi[:, None], mi[None, :])
    m4 = m ** 4
    # rescale so w2 = msk*soft*cnt fits fp8e4m3 range; the global scale
    # cancels in the intra-cluster L2 normalization on the host
    msk_full = (m4 / m4.max()).astype(np.float32)            # [H,W]

    xn_pad = np.zeros((C, H + 2, W), np.float32)
    xn_pad[:, 1:H + 1] = xn
    msk_pad = np.zeros((H + 2, W), np.float32)
    msk_pad[1:H + 1] = msk_full

    # packed small inputs
    cwtb = conv_w.T.astype(np.float32).reshape(CT, 128, K)
    cwtb = np.ascontiguousarray(cwtb.transpose(1, 0, 2)).reshape(128, CT * K)
    small8 = np.zeros((128, CT * K + 8), np.float32)
    small8[:, 0:CT * K] = cwtb
    small8[:, CT * K:] = 1.0
    small8 = small8.astype(fp8np)
    identb = np.eye(128, dtype=np.float32)
    mstack = np.concatenate([np.eye(K), np.eye(K)], 0).astype(np.float32)
    slot = np.array(SLOT_TILES)

    in_maps = []
    for core in range(M):
        r0 = core * RPC
        slab = np.ascontiguousarray(
            xn_pad[:, r0:r0 + RPC + 2, :].reshape(C, Ls))
        mskc = msk_pad[r0:r0 + RPC + 2].reshape(Ls).copy()
        mskc[0:W] = 0.0
        mskc[(RPC + 1) * W:] = 0.0                           # halo rows -> 0
        xnT = np.ascontiguousarray(slab.T).astype(bf16np)    # [Ls, C]
        # permute l-tiles into VLAD slot order
        xnT_perm = np.ascontiguousarray(
            xnT.reshape(NT, 128, C)[slot].reshape(Ls, C))
        smallb = np.zeros((128, 128 + K + NT), np.float32)
        smallb[:, 0:128] = identb
        smallb[:, 128:128 + K] = mstack
        smallb[:, 128 + K:] = mskc.reshape(NT, 128).T
        in_maps.append({
            "xnb": slab.astype(fp8np),
            "xnt": xnT_perm,
            "smallb": smallb.astype(bf16np),
            "small8": small8,
        })
    return in_maps


def _ensure_ntff_hook():
    """Install the axon NTFF profile hook if the image's antenv lacks it."""
    import types
    try:
        from antenv.axon_hooks import get_axon_ntff_profile_hook  # noqa: F401
        return
    except ImportError:
        pass
    if "/root/.axon_site" not in sys.path:
        sys.path.insert(0, "/root/.axon_site")
    from trn_agent_boot.trn_boot import _ntff_profile_via_ctypes
    hook = _ntff_profile_via_ctypes("/opt/axon/libaxon_pjrt.so")
    mod = types.ModuleType("antenv.axon_hooks")
    mod.get_axon_ntff_profile_hook = lambda: hook
    mod.set_axon_ntff_profile_hook = lambda h: None
    import antenv
    antenv.axon_hooks = mod
    sys.modules["antenv.axon_hooks"] = mod


def _install_neff_cache():
    """Cache compiled NEFFs across processes, keyed by BIR content hash."""
    import hashlib
    import shutil
    import concourse.bass2jax as b2j

    orig = b2j.compile_bir_kernel
    if getattr(orig, "_neff_cached", False):
        return

    def cached(bir_json, tmpdir, neff_name="file.neff"):
        h = hashlib.sha256(
            bir_json if isinstance(bir_json, bytes) else bir_json.encode()
        ).hexdigest()[:24]
        cdir = "/tmp/neff_cache"
        os.makedirs(cdir, exist_ok=True)
        cpath = os.path.join(cdir, h + ".neff")
        if os.path.exists(cpath):
            dst = os.path.join(tmpdir, neff_name)
            os.makedirs(tmpdir, exist_ok=True)
            shutil.copy(cpath, dst)
            return dst
        out = orig(bir_json, tmpdir, neff_name=neff_name)
        shutil.copy(out, cpath)
        return out

    cached._neff_cached = True
    b2j.compile_bir_kernel = cached


def kernel(x, conv_w, centroids):
    import concourse.bass_utils as bu
    from concourse.bass_utils import run_bass_kernel_spmd
    _install_neff_cache()
    if TRACE:
        _ensure_ntff_hook()
        bu.upload_artifacts = lambda tmpdir: "local://" + tmpdir

    if "nc" not in _CACHE:
        _CACHE["nc"] = _build_nc()
    nc = _CACHE["nc"]
    in_maps = _host_prep(np.asarray(x), np.asarray(conv_w), np.asarray(centroids))
    res = run_bass_kernel_spmd(nc, in_maps, list(range(M)), trace=TRACE)
    _CACHE["last"] = res
    red = np.zeros((128, C + 1), np.float64)
    for r in res.results:
        red += np.asarray(r["y"], dtype=np.float64)
    redk = red[0:64] + red[64:128]                           # [K, C+1]
    vlad = redk[:, :C] - redk[:, C:C + 1] * np.asarray(centroids, np.float64)
    vlad /= np.maximum(np.sqrt((vlad ** 2).sum(1))[:, None], 1e-12)
    v = vlad.reshape(1, K * C)
    v /= np.maximum(np.sqrt((v ** 2).sum()), 1e-12)
    return v.astype(np.float32)
